# revision 6
# baseline (speedup 1.0000x reference)
"""Trainium2 Bass kernel for BinaryDecoderV2 — v3 (2x4 sharding).

Same pipeline as v2.5 (bit-packed weights, 3-pass nibble unpack, diagonal
pack matmuls, fp8 DoubleRow mains) but sharded 2-way over batch x 4-way over
out_features: per-core latent halves to 8.4MB, weights bits 2.1MB, true_sum
2.1MB -> ~12.7MB HBM/core instead of 20.2MB. PE main work is unchanged
(same MACs/core); pack work doubles (weight slice is 2x wider) but stays
far under the DMA roofline.

Core c: out-shard c%4 (256 outputs), batch-shard c//4 (1024 rows).
"""

import numpy as np
import ml_dtypes

IN_FEATURES = 8192
OUT_FEATURES = 1024
N_BITS = 8
BATCH = 2048
N_CORES = 8
OSH = 4                        # out-feature shards
BSH = 2                        # batch shards
OPC = OUT_FEATURES // OSH      # 256 outputs per core
BC = BATCH // BSH              # 1024 batch rows per core
KP = 128
KT = IN_FEATURES // KP         # 64 k-subtiles
DKT = KT // 2                  # 32 DoubleRow k-tiles
NRND = 32                      # pack rounds (2 kt each)
MEGA = 4                       # unpack mega-rounds (16 kt each)
NCHUNK = 512
NCH = BC // NCHUNK             # 2 batch chunks per core
_LCH = [16, 16, 16, 8, 4, 4]
LCH_START = {}
_s = 0
for _i, _n in enumerate(_LCH):
    LCH_START[_s] = (_i, _n)
    _s += _n
assert _s == KT
POWERS = [1.0, 2.0, 4.0, 8.0, 16.0, 32.0, 64.0, -128.0]
C_PLANES = [-0.5, -8.0]   # lo nibble, hi (sign-flipped) nibble
SCALE = 2.0 ** N_BITS - 1.0

_CACHE: dict = {}


def _build():
    import concourse.bacc as bacc
    import concourse.mybir as mybir
    from concourse import tile

    f8e4 = mybir.dt.float8e4
    u8 = mybir.dt.uint8
    f32 = mybir.dt.float32
    Act = mybir.ActivationFunctionType
    Alu = mybir.AluOpType
    PM = mybir.MatmulPerfMode

    nc = bacc.Bacc("TRN2", target_bir_lowering=False, debug=False,
                   num_devices=N_CORES)

    latq = nc.dram_tensor("latq", [128, KT, BC], f8e4,
                          kind="ExternalInput")
    w8 = nc.dram_tensor("w8", [128, KT, OPC], u8, kind="ExternalInput")
    tq = nc.dram_tensor("tq", [128, 2, N_BITS, BC], f8e4,
                        kind="ExternalInput")
    dg = nc.dram_tensor("dg", [128, N_BITS, 128], f8e4,
                        kind="ExternalInput")
    cp = nc.dram_tensor("cp", [128, 2, 128], f8e4, kind="ExternalInput")
    partials = nc.dram_tensor("partials", [128, 4], f32,
                              kind="ExternalOutput")

    with tile.TileContext(nc) as tc:
        with (
            tc.tile_pool(name="w8p", bufs=1) as w8_pool,
            tc.tile_pool(name="tsp", bufs=1) as tsp_pool,
            tc.tile_pool(name="cst", bufs=1) as cst_pool,
            tc.tile_pool(name="tp", bufs=2) as t_pool,
            tc.tile_pool(name="iw", bufs=1) as iw_pool,
            tc.tile_pool(name="lat", bufs=4) as lat_pool,
            tc.tile_pool(name="loss", bufs=1) as loss_pool,
            tc.tile_pool(name="ps", bufs=1, space="PSUM") as psum_pool,
            tc.tile_pool(name="pk", bufs=2, space="PSUM") as pk_pool,
        ):
            dgt = cst_pool.tile([128, N_BITS, 128], f8e4, name="dgt",
                                tag="dgt")
            nc.sync.dma_start(dgt[:], dg[:])
            cpt = cst_pool.tile([128, 2, 128], f8e4, name="cpt", tag="cpt")
            nc.sync.dma_start(cpt[:], cp[:])

            w8t = w8_pool.tile([128, KT, OPC], u8)
            tp = tsp_pool.tile([128, 2, N_BITS, BC], f8e4)
            nc.sync.dma_start(w8t[:, 0:16, :], w8[:, 0:16, :])
            nc.sync.dma_start(tp[:, 0, :, :], tq[:, 0, :, :])
            nc.sync.dma_start(w8t[:, 16:32, :], w8[:, 16:32, :])
            nc.sync.dma_start(tp[:, 1, :, :], tq[:, 1, :, :])
            nc.sync.dma_start(w8t[:, 32:48, :], w8[:, 32:48, :])
            nc.sync.dma_start(w8t[:, 48:64, :], w8[:, 48:64, :])

            # ---- int_sum into 4 psums: index = oh*2 + ch ----
            psums = [psum_pool.tile([128, NCHUNK], f32, name=f"ps{i}",
                                    tag=f"ps{i}") for i in range(4)]
            for oh in range(2):
                for bp in range(4):
                    for ch in range(NCH):
                        nc.tensor.matmul(
                            psums[oh * NCH + ch][:],
                            dgt[:, 2 * bp:2 * bp + 2, :],
                            tp[:, oh, 2 * bp:2 * bp + 2,
                               ch * NCHUNK:(ch + 1) * NCHUNK],
                            start=(bp == 0), stop=False,
                            perf_mode=PM.DoubleRow)

            # ---- weight pipeline + main matmul stream ----
            iwts = [iw_pool.tile([128, 2, OPC], f8e4, name=f"iw{r}",
                                 tag=f"iw{r}") for r in range(NRND)]
            out_t = loss_pool.tile([128, 4], f32, name="out_t",
                                   tag="out_t")
            lts = {}

            def issue_lat(kt):
                if kt in LCH_START:
                    q, n = LCH_START[kt]
                    lt = lat_pool.tile([128, n, BC], f8e4,
                                       name=f"lt{q}", tag="lat")
                    nc.sync.dma_start(lt[:], latq[:, kt:kt + n, :])
                    lts[kt] = (lt, kt)

            tts = {}

            def pack_round(r):
                # unpack 16 kt worth: 2 nibble planes [128, 2, 16*OPC]
                # (host pre-XORs 0x80, so hi' = x>>4 and the -128 constant
                # folds into the cast bias)
                mr, rr = divmod(r, 8)
                if rr == 0:
                    tt = t_pool.tile([128, 2, 16 * OPC], f8e4,
                                     name=f"tt{mr}", tag="tt")
                    w8s = w8t[:, 16 * mr:16 * (mr + 1), :]
                    nc.vector.tensor_scalar(tt[:, 0, :].bitcast(u8), w8s,
                                            15, None, Alu.bitwise_and)
                    nc.vector.tensor_scalar(tt[:, 1, :].bitcast(u8), w8s,
                                            4, None,
                                            Alu.logical_shift_right)
                    tts[mr] = tt
                tt = tts[mr]
                pkb = pk_pool.tile([128, NCHUNK], f32, name=f"pk{r}",
                                   tag="pk")
                nc.tensor.matmul(
                    pkb[:], cpt[:, 0:2, :],
                    tt[:, 0:2, rr * NCHUNK:(rr + 1) * NCHUNK],
                    start=True, stop=True, perf_mode=PM.DoubleRow)
                # intw = 1024*(-(n_lo+16*n_hi')/1024) + 128 = -int_w
                nc.scalar.activation(iwts[r][:], pkb[:], Act.Copy,
                                     scale=1024.0, bias=128.0)

            # pack/cast pipelined 2 rounds ahead of the main matmuls
            pack_round(0)
            pack_round(1)
            for r in range(NRND):
                dkt = r
                kt0 = 2 * dkt
                issue_lat(kt0)
                cur = lts[max(s for s in lts if s <= kt0)]
                lt, base = cur
                a = kt0 - base
                last = (dkt == DKT - 1)
                for oh in range(2):
                    lhsT = iwts[r][:, :, oh * 128:(oh + 1) * 128]
                    for ch in range(NCH):
                        nc.tensor.matmul(
                            psums[oh * NCH + ch][:], lhsT,
                            lt[:, a:a + 2,
                               ch * NCHUNK:(ch + 1) * NCHUNK],
                            start=False, stop=last,
                            perf_mode=PM.DoubleRow)
                        if last:
                            i4 = oh * NCH + ch
                            d2 = loss_pool.tile(
                                [128, NCHUNK], f32, name=f"d2_{i4}",
                                tag=f"d2_{i4}")
                            nc.scalar.activation(
                                d2[:], psums[i4][:], Act.Square,
                                accum_out=out_t[:, i4:i4 + 1])
                if r + 2 < NRND:
                    pack_round(r + 2)

            nc.sync.dma_start(partials[:], out_t[:])

    nc.compile()
    return nc


def _get_nc():
    if "nc" not in _CACHE:
        _CACHE["nc"] = _build()
    return _CACHE["nc"]


def make_in_maps(latent: np.ndarray, true_sum: np.ndarray,
                 weight: np.ndarray) -> list:
    f8 = ml_dtypes.float8_e4m3fn

    # latq per batch shard: latq[p, kt, n] = latent[sb*BC + n, kt*128 + p]
    lat8 = latent.astype(f8)
    latqs = []
    for sb in range(BSH):
        ls = lat8[sb * BC:(sb + 1) * BC, :]
        latqs.append(np.ascontiguousarray(
            ls.T.reshape(KT, KP, BC).transpose(1, 0, 2)))

    bits = (weight > 0).astype(np.uint8).reshape(IN_FEATURES,
                                                 OUT_FEATURES, N_BITS)
    shifts = (1 << np.arange(N_BITS, dtype=np.uint16))
    bytes_ko = ((bits.astype(np.uint16) * shifts).sum(-1)
                .astype(np.uint8) ^ 0x80)   # flip sign bit: -128 -> cast bias
    w8s = []
    for so in range(OSH):
        wcol = bytes_ko[:, so * OPC:(so + 1) * OPC]
        w8s.append(np.ascontiguousarray(
            wcol.reshape(KT, KP, OPC).transpose(1, 0, 2)))

    dg = np.zeros((128, N_BITS, 128), dtype=np.float32)
    for b in range(N_BITS):
        np.fill_diagonal(dg[:, b, :], POWERS[b])
    dg8 = dg.astype(f8)
    cpm = np.zeros((128, 2, 128), dtype=np.float32)
    for j in range(2):
        np.fill_diagonal(cpm[:, j, :], C_PLANES[j])
    cp8 = cpm.astype(f8)

    ts8 = true_sum.astype(f8)
    in_maps = []
    for c in range(N_CORES):
        so, sb = c % OSH, c // OSH
        # tq[o128, oh, b, n] = true_sum[sb*BC+n, (so*256 + oh*128 + o128)*8 + b]
        T = ts8[sb * BC:(sb + 1) * BC,
                so * OPC * N_BITS:(so + 1) * OPC * N_BITS]
        t5 = T.reshape(BC, 2, 128, N_BITS)       # [n, oh, o128, b]
        tql = np.ascontiguousarray(t5.transpose(2, 1, 3, 0))
        in_maps.append({"latq": latqs[sb], "w8": w8s[so], "tq": tql,
                        "dg": dg8, "cp": cp8})
    return in_maps


def kernel(latent: np.ndarray, true_sum: np.ndarray,
           weight: np.ndarray) -> np.ndarray:
    from concourse.bass_utils import run_bass_kernel_spmd

    nc = _get_nc()
    in_maps = make_in_maps(latent, true_sum, weight)
    res = run_bass_kernel_spmd(nc, in_maps, list(range(N_CORES)))

    total = 0.0
    for c in range(N_CORES):
        total += float(res.results[c]["partials"].astype(np.float64).sum())
    loss = total / (BATCH * OUT_FEATURES) / (SCALE * SCALE)
    return np.array(loss, dtype=np.float32)
